# revision 30
# baseline (speedup 1.0000x reference)
"""Trainium2 Bass kernel for ContinuousAttention (self-keyed RoPE attention,
strictly-causal masked scores, no softmax).

Reference computation (B=2, NH=16, T=2048, N=256, fp32):
    QR = rope(Q)                      # interleaved-pair RoPE, freqs quantized in pairs
    S  = QR @ QR^T                    # per (b, h); K input is unused by the module
    O  = (S * strict_causal_mask) @ V

Sharding: 32 (b*nh) heads over 8 NeuronCores, 4 heads per core; no
communication.  Each core runs an identical program on its head slice.

v6 design — chunked linear attention (no softmax => scores are linear):
    O_i = QR_i @ H_{<i} + (causal diagonal blocks) @ V,   H += QR_i^T V_i
with a running state H (256x256) accumulated in fp32 PSUM across each head.
PE work is ~2*T*N^2 + ~2.5*T*C*N per head, ~2.7x less than dense-causal.

Superchunks of 256 rows (2 chunks i0, i1) keep the PSUM-drain op count low
(vector/scalar are the only engines that may read PSUM and each drain op has
a few-hundred-ns fixed cost):
  - one [128, 384] score PSUM bank holds diag(i0) | dense(i1,i0) | diag(i1),
    drained by a single mask-multiply-cast (mask = strict|ones|strict),
  - one [128, 512] O PSUM bank holds O_i0 | O_i1, drained by a single cast,
  - one H copy per superchunk; O_i1's missing chunk-i0 term comes from the
    dense block instead of H.
PSUM has_written semantics: start=True clears the accumulate bits of the
WHOLE bank, so only the first matmul targeting a bank uses it; later groups
in the same bank open with start=False (overwrite-where-unset).

Host ships QR pre-rotated in both (n, t) and p-major (t, n) fp16 layouts and
V p-major fp16; all device DMAs are contiguous 2D copies.  Output is fp16
p-major, unpacked on host.  Two heads are interleaved superchunk-by-
superchunk so every drain has a full other-head superchunk of latency cover.
"""

import math
import sys

import numpy as np

if "/opt/trn_rl_repo" not in sys.path:
    sys.path.insert(0, "/opt/trn_rl_repo")

import concourse.bass as bass
import concourse.mybir as mybir
import concourse.tile as tile
from concourse.bass_utils import run_bass_kernel_spmd

B, NH, T, N = 2, 16, 2048, 256
THETA = 2 ** 16
N_CORES = 8
H_PER_CORE = (B * NH) // N_CORES

F32 = mybir.dt.float32
FP16 = mybir.dt.float16
MULT = mybir.AluOpType.mult
HF = np.float16


def _split_overloaded_waits(nc, max_waits=1):
    """walrus in this container rejects >1 sync-wait per instruction; move
    extra waits onto preceding same-engine NoOps (semantically identical)."""
    n_split = 0
    for f in nc.m.functions:
        for bb in f.blocks:
            new_list = []
            changed = False
            for ins in bb.instructions:
                si = getattr(ins, "sync_info", None)
                if si is not None and len(si.on_wait) > max_waits:
                    waits = list(si.on_wait)
                    extra, keep = waits[:-max_waits], waits[-max_waits:]
                    k = 0
                    while extra:
                        chunk, extra = extra[:max_waits], extra[max_waits:]
                        nop = mybir.InstNoOp(
                            name=f"{ins.name}_wsplit{k}", ins=[], outs=[]
                        )
                        nop.engine = ins.engine
                        nop.sync_info = mybir.SyncInfo(on_wait=chunk, on_update=[])
                        new_list.append(nop)
                        k += 1
                    ins.sync_info = mybir.SyncInfo(
                        on_wait=keep, on_update=list(si.on_update)
                    )
                    changed = True
                    n_split += 1
                new_list.append(ins)
            if changed:
                bb.instructions = new_list
    return n_split


def rope_tables(t=T, n=N, dtype=np.float32):
    """cos table and sign-folded sin table, natural (t, n) layout."""
    idx = np.floor(np.arange(n, dtype=dtype) / dtype(2.0)) * dtype(2.0)
    freqs = (
        dtype(1.0) / (dtype(THETA) ** (idx / dtype(n))) / dtype(2.0 * math.pi)
    ).astype(dtype)
    phases = np.arange(t, dtype=dtype)[:, None] * freqs[None, :]
    ph = (phases % dtype(1.0)) * dtype(2.0 * math.pi)
    cos = np.cos(ph).astype(dtype)
    sin = np.sin(ph).astype(dtype)
    sin_a = sin.copy()
    sin_a[:, 0::2] *= dtype(-1.0)  # fold the rotate-pair sign into sin
    return cos, sin_a


def build_nc(h_per_core=H_PER_CORE, t=T, n=N, waitsplit=True):
    assert n == 256 and t % 256 == 0
    nt = t // 128   # 128-row chunks per head (16)
    ns = t // 256   # superchunks per head (8)
    nc = bass.Bass("TRN2", target_bir_lowering=False, debug=False)

    # qrt: rotated Q, (n, t) layout, two 128-partition n-halves
    qrtd = nc.dram_tensor(
        "qrt", [h_per_core, 2, 128, t], FP16, kind="ExternalInput"
    ).ap()
    # qtn: rotated Q, p-major packed (t, n): qtn[h, p, ci*n+m] = QR[h, ci*128+p, m]
    qtnd = nc.dram_tensor(
        "qtn", [h_per_core, 128, nt * n], FP16, kind="ExternalInput"
    ).ap()
    # v: p-major packed like qtn
    vd = nc.dram_tensor(
        "v", [h_per_core, 128, nt * n], FP16, kind="ExternalInput"
    ).ap()
    # o: p-major packed fp16; host unpacks + casts
    od = nc.dram_tensor(
        "o", [h_per_core, 128, nt * n], FP16, kind="ExternalOutput"
    ).ap()

    with tile.TileContext(nc) as tc:
        with (
            tc.tile_pool(name="const", bufs=1) as cpool,
            tc.tile_pool(name="qrt", bufs=4) as qpool,
            tc.tile_pool(name="qtn", bufs=4) as qnpool,
            tc.tile_pool(name="vh", bufs=4) as vpool,
            tc.tile_pool(name="hs", bufs=2) as hspool,
            tc.tile_pool(name="sts", bufs=4) as stspool,
            tc.tile_pool(name="ohs", bufs=4) as ohpool,
            tc.tile_pool(name="hp", bufs=2, space="PSUM") as hpool,
            tc.tile_pool(name="op", bufs=4, space="PSUM") as opool,
            tc.tile_pool(name="sp", bufs=2, space="PSUM") as sppool,
        ):
            # mask for one superchunk's score drain, (s, t') orientation:
            # [0:128]  = strict upper (diag i0), [128:256] = ones (dense),
            # [256:384]= strict upper (diag i1), [384:512] = ones (warmup).
            mask = cpool.tile([128, 512], F32)
            nc.gpsimd.memset(mask, 1.0)
            for c0 in (0, 256):
                nc.gpsimd.affine_select(
                    out=mask[:, c0:c0 + 128],
                    in_=mask[:, c0:c0 + 128],
                    compare_op=mybir.AluOpType.is_ge,
                    fill=0.0,
                    base=-1,
                    pattern=[[1, 128]],
                    channel_multiplier=-1,
                )

            # HAM warmup: dummy fp32 PE activity while head 0's input DMAs
            # are in flight starts the un-throttle clock early.
            for _ in range(3):
                warm = opool.tile([128, 512], F32, tag="op", name="warm")
                nc.tensor.matmul(
                    warm, lhsT=mask[:, 0:128], rhs=mask,
                    start=True, stop=True,
                )

            qrt = {}
            qtn = {}
            vh = {}
            hp = {}
            hs = {}
            dr = [0]

            def emit_loads(all_heads):
                """All input DMAs upfront, pair-ordered + consumption-ordered:
                pair 0's whole working set streams before pair 1 touches the
                rings.  Dedicated rings (sync: qrt, scalar: v+qtn interleaved,
                gpsimd: outputs) keep the critical first ~10us parallel."""
                for h in all_heads:
                    qrt[h] = [
                        qpool.tile(
                            [128, t], FP16, tag=f"qrt{c}", name=f"qrt{c}_{h}"
                        )
                        for c in range(2)
                    ]
                    qtn[h] = qnpool.tile(
                        [128, nt * n], FP16, tag="qtn", name=f"qtn{h}"
                    )
                    vh[h] = vpool.tile([128, nt * n], FP16, tag="vh", name=f"vh{h}")
                pairs = [all_heads[i:i + 2] for i in range(0, len(all_heads), 2)]
                for ph in pairs:
                    for s in range(4):
                        tsl = slice(s * (t // 4), (s + 1) * (t // 4))
                        for h in ph:
                            for c in range(2):
                                nc.sync.dma_start(
                                    out=qrt[h][c][:, tsl], in_=qrtd[h, c][:, tsl]
                                )
                    for s in range(8):  # fine v/qtn segs: superchunk 0's data
                        vsl = slice(s * (nt * n // 8), (s + 1) * (nt * n // 8))
                        for h in ph:    # lands ~1us in
                            nc.scalar.dma_start(out=vh[h][:, vsl], in_=vd[h][:, vsl])
                            nc.scalar.dma_start(out=qtn[h][:, vsl], in_=qtnd[h][:, vsl])

            def emit_scores(h, sc):
                """diag(i0) | dense(i1<-i0) | diag(i1) into one PSUM bank,
                single fused mask-mult-cast drain."""
                i0s = slice(sc * 256, sc * 256 + 128)
                i1s = slice(sc * 256 + 128, sc * 256 + 256)
                scs = slice(sc * 256, sc * 256 + 256)
                sp = sppool.tile([128, 384], F32, name="sp")
                first = True
                # diag0|dense share lhsT=QR_i0^T: one 256-wide rhs covers both
                for (osl, ls, rs) in (
                    (slice(0, 256), i0s, scs),
                    (slice(256, 384), i1s, i1s),
                ):
                    for c in range(2):
                        nc.tensor.matmul(
                            sp[:, osl],
                            lhsT=qrt[h][c][:, ls],
                            rhs=qrt[h][c][:, rs],
                            start=first, stop=(c == 1),
                            skip_group_check=True,
                        )
                        first = False
                sts = stspool.tile([128, 384], FP16, name="sts")
                nc.vector.tensor_tensor(
                    out=sts, in0=sp, in1=mask[:, 0:384], op=MULT
                )
                return sts

            def emit_out(h, sc, sts, last_head=False):
                """inter + intra2 for both chunks into one O bank; state for
                both chunks; single O cast + DMA; single H copy."""
                i0c, i1c = 2 * sc, 2 * sc + 1
                m0 = slice(i0c * n, (i0c + 1) * n)
                m1 = slice(i1c * n, (i1c + 1) * n)
                i0s = slice(sc * 256, sc * 256 + 128)
                i1s = slice(sc * 256 + 128, sc * 256 + 256)
                op = opool.tile([128, 512], F32, name="op", tag="op")
                first = True
                if sc > 0:  # inter: O_i += QR_i @ H_{<superchunk}
                    for (osl, csl) in ((slice(0, 256), i0s), (slice(256, 512), i1s)):
                        for c in range(2):
                            nc.tensor.matmul(
                                op[:, osl],
                                lhsT=qrt[h][c][:, csl],
                                rhs=hs[h][:, c * 256:(c + 1) * 256],
                                start=first, stop=False,
                                skip_group_check=True,
                            )
                            first = False
                # intra2: diagonal score blocks @ V
                nc.tensor.matmul(
                    op[:, 0:256], lhsT=sts[:, 0:128], rhs=vh[h][:, m0],
                    start=first, stop=(sc > 0), skip_group_check=True,
                )
                nc.tensor.matmul(
                    op[:, 256:512], lhsT=sts[:, 128:256], rhs=vh[h][:, m0],
                    start=False, stop=False, skip_group_check=True,
                )
                nc.tensor.matmul(
                    op[:, 256:512], lhsT=sts[:, 256:384], rhs=vh[h][:, m1],
                    start=False, stop=True, skip_group_check=True,
                )
                oh = ohpool.tile([128, 512], FP16, name="oh")
                if dr[0] % 2 == 0:
                    nc.scalar.copy(out=oh, in_=op)
                else:
                    nc.vector.tensor_copy(out=oh, in_=op)
                nc.gpsimd.dma_start(
                    out=od[h][:, i0c * n:(i1c + 1) * n], in_=oh
                )
                if sc < ns - 1:
                    # state: H += QR_i0^T V_i0 + QR_i1^T V_i1 (open fp32
                    # accumulation across the head; only the head's first
                    # matmul may use start=True — bank-wide bit clear)
                    for ci, msl in ((i0c, m0), (i1c, m1)):
                        for c in range(2):
                            nc.tensor.matmul(
                                hp[h][:, c * 256:(c + 1) * 256],
                                lhsT=qtn[h][:, ci * n + c * 128: ci * n + (c + 1) * 128],
                                rhs=vh[h][:, msl],
                                start=(ci == 0 and c == 0),
                                stop=(sc == ns - 2 and ci == i1c),
                                skip_group_check=True,
                            )
                    # H-copy on the engine not doing this head's O cast
                    if dr[0] % 2 == 0:
                        nc.vector.tensor_copy(out=hs[h], in_=hp[h])
                    else:
                        nc.scalar.copy(out=hs[h], in_=hp[h])
                dr[0] += 1

            # all four heads' inputs fit in SBUF: load everything upfront so
            # pair 1's data streams in while pair 0 computes, on pure-DMA rings
            emit_loads(list(range(h_per_core)))
            for pair in range(h_per_core // 2):
                heads = (2 * pair, 2 * pair + 1)
                for h in heads:
                    hp[h] = hpool.tile([128, 512], F32, tag="hp", name=f"hp{h}")
                    hs[h] = hspool.tile([128, 512], FP16, tag="hs", name=f"hs{h}")
                # scores run one superchunk ahead of the out/state stage so
                # every score-drain has a full stage of PE cover
                cur = {h: emit_scores(h, 0) for h in heads}
                for sc in range(ns):
                    nxt = {}
                    for h in heads:
                        if sc + 1 < ns:
                            nxt[h] = emit_scores(h, sc + 1)
                        emit_out(h, sc, cur[h])
                    cur = nxt

    if waitsplit:
        _split_overloaded_waits(nc)
    return nc


_NC_CACHE = {}


def get_nc(h_per_core=H_PER_CORE, t=T, n=N):
    key = (h_per_core, t, n)
    if key not in _NC_CACHE:
        _NC_CACHE[key] = build_nc(h_per_core, t, n)
    return _NC_CACHE[key]


def make_in_maps(Q, V, n_cores=N_CORES):
    b, nh, t, n = Q.shape
    h_per_core = (b * nh) // n_cores
    nt = t // 128
    qf = np.asarray(Q, dtype=np.float32).reshape(b * nh, t, n)
    vf = np.asarray(V, dtype=np.float32).reshape(b * nh, t, n)
    # RoPE on host in fp32 (input prep, like the layout transposes):
    # qr = q * cos + pairswap(q) * sign-folded-sin
    qsw = qf.reshape(b * nh, t, n // 2, 2)[..., ::-1].reshape(b * nh, t, n)
    cos, sin_a = rope_tables(t, n)
    qr = (qf * cos + qsw * sin_a).astype(HF)
    # (n, t) layout, n-halves split for direct 128-partition DMAs
    qrtb = np.ascontiguousarray(
        qr.transpose(0, 2, 1).reshape(b * nh, 2, 128, t)
    )

    def pmajor(x):  # [h, t, n] -> [h, 128, nt*n] with x[h, ci*128+p, m]
        return np.ascontiguousarray(
            x.reshape(b * nh, nt, 128, n).transpose(0, 2, 1, 3)
        ).reshape(b * nh, 128, nt * n)

    qtnb = pmajor(qr)
    vb = pmajor(vf.astype(HF))
    in_maps = []
    for c in range(n_cores):
        sl = slice(c * h_per_core, (c + 1) * h_per_core)
        in_maps.append(
            {
                "qrt": np.ascontiguousarray(qrtb[sl]),
                "qtn": np.ascontiguousarray(qtnb[sl]),
                "v": np.ascontiguousarray(vb[sl]),
            }
        )
    return in_maps


def unpack_out(outs, b, nh, t, n):
    """[cores][h, 128, nt*n] p-major fp16 -> (b, nh, t, n) fp32."""
    nt = t // 128
    full = np.concatenate(outs, axis=0)  # (b*nh, 128, nt*n)
    full = full.reshape(b * nh, 128, nt, n).transpose(0, 2, 1, 3)
    return np.ascontiguousarray(full).reshape(b, nh, t, n).astype(np.float32)


def kernel(Q, K, V):
    """Full-input entry point: Q, K, V are (B, NH, T, N) float32 numpy arrays.
    K is unused (the module self-keys attention on rotated Q)."""
    Q = np.asarray(Q)
    V = np.asarray(V)
    b, nh, t, n = Q.shape
    nc = get_nc((b * nh) // N_CORES, t, n)
    in_maps = make_in_maps(Q, V, N_CORES)
    res = None
    last_err = None
    for attempt in range(3):  # retry transient device/runtime failures
        try:
            res = run_bass_kernel_spmd(
                nc, in_maps, core_ids=list(range(N_CORES)), trace=False
            )
            break
        except Exception as e:  # e.g. NRT_EXEC_UNIT_UNRECOVERABLE after a
            last_err = e  # wedged prior run; a clean retry usually recovers
            import time as _time

            _time.sleep(2.0 * (attempt + 1))
    if res is None:
        raise last_err
    outs = [res.results[c]["o"] for c in range(N_CORES)]
    return unpack_out(outs, b, nh, t, n)


# revision 34
# speedup vs baseline: 1.3818x; 1.3818x over previous
"""Trainium2 Bass kernel for ContinuousAttention (self-keyed RoPE attention,
strictly-causal masked scores, no softmax).

Reference computation (B=2, NH=16, T=2048, N=256, fp32):
    QR = rope(Q)                      # interleaved-pair RoPE, freqs quantized in pairs
    S  = QR @ QR^T                    # per (b, h); K input is unused by the module
    O  = (S * strict_causal_mask) @ V

Sharding: 32 (b*nh) heads over 8 NeuronCores, 4 heads per core; no
communication.  Each core runs an identical program on its head slice.

v6 design — chunked linear attention (no softmax => scores are linear):
    O_i = QR_i @ H_{<i} + (causal diagonal blocks) @ V,   H += QR_i^T V_i
with a running state H (256x256) accumulated in fp32 PSUM across each head.
PE work is ~2*T*N^2 + ~2.5*T*C*N per head, ~2.7x less than dense-causal.

Superchunks of 256 rows (2 chunks i0, i1) keep the PSUM-drain op count low
(vector/scalar are the only engines that may read PSUM and each drain op has
a few-hundred-ns fixed cost):
  - one [128, 384] score PSUM bank holds diag(i0) | dense(i1,i0) | diag(i1),
    drained by a single mask-multiply-cast (mask = strict|ones|strict),
  - one [128, 512] O PSUM bank holds O_i0 | O_i1, drained by a single cast,
  - one H copy per superchunk; O_i1's missing chunk-i0 term comes from the
    dense block instead of H.
PSUM has_written semantics: start=True clears the accumulate bits of the
WHOLE bank, so only the first matmul targeting a bank uses it; later groups
in the same bank open with start=False (overwrite-where-unset).

Host ships QR pre-rotated in both (n, t) and p-major (t, n) fp16 layouts and
V p-major fp16; all device DMAs are contiguous 2D copies.  Output is fp16
p-major, unpacked on host.  Two heads are interleaved superchunk-by-
superchunk so every drain has a full other-head superchunk of latency cover.
"""

import math
import sys

import numpy as np

if "/opt/trn_rl_repo" not in sys.path:
    sys.path.insert(0, "/opt/trn_rl_repo")

import concourse.bass as bass
import concourse.mybir as mybir
import concourse.tile as tile
from concourse.bass_utils import run_bass_kernel_spmd

B, NH, T, N = 2, 16, 2048, 256
THETA = 2 ** 16
N_CORES = 8
H_PER_CORE = (B * NH) // N_CORES

F32 = mybir.dt.float32
FP16 = mybir.dt.float16
MULT = mybir.AluOpType.mult
HF = np.float16


def _split_overloaded_waits(nc, max_waits=1):
    """walrus in this container rejects >1 sync-wait per instruction; move
    extra waits onto preceding same-engine NoOps (semantically identical)."""
    n_split = 0
    for f in nc.m.functions:
        for bb in f.blocks:
            new_list = []
            changed = False
            for ins in bb.instructions:
                si = getattr(ins, "sync_info", None)
                if si is not None and len(si.on_wait) > max_waits:
                    waits = list(si.on_wait)
                    extra, keep = waits[:-max_waits], waits[-max_waits:]
                    k = 0
                    while extra:
                        chunk, extra = extra[:max_waits], extra[max_waits:]
                        nop = mybir.InstNoOp(
                            name=f"{ins.name}_wsplit{k}", ins=[], outs=[]
                        )
                        nop.engine = ins.engine
                        nop.sync_info = mybir.SyncInfo(on_wait=chunk, on_update=[])
                        new_list.append(nop)
                        k += 1
                    ins.sync_info = mybir.SyncInfo(
                        on_wait=keep, on_update=list(si.on_update)
                    )
                    changed = True
                    n_split += 1
                new_list.append(ins)
            if changed:
                bb.instructions = new_list
    return n_split


def rope_tables(t=T, n=N, dtype=np.float32):
    """cos table and sign-folded sin table, natural (t, n) layout."""
    idx = np.floor(np.arange(n, dtype=dtype) / dtype(2.0)) * dtype(2.0)
    freqs = (
        dtype(1.0) / (dtype(THETA) ** (idx / dtype(n))) / dtype(2.0 * math.pi)
    ).astype(dtype)
    phases = np.arange(t, dtype=dtype)[:, None] * freqs[None, :]
    ph = (phases % dtype(1.0)) * dtype(2.0 * math.pi)
    cos = np.cos(ph).astype(dtype)
    sin = np.sin(ph).astype(dtype)
    sin_a = sin.copy()
    sin_a[:, 0::2] *= dtype(-1.0)  # fold the rotate-pair sign into sin
    return cos, sin_a


def build_nc(h_per_core=H_PER_CORE, t=T, n=N, waitsplit=True):
    assert n == 256 and t % 256 == 0
    nt = t // 128   # 128-row chunks per head (16)
    ns = t // 256   # superchunks per head (8)
    nc = bass.Bass("TRN2", target_bir_lowering=False, debug=False)

    # qrt: rotated Q, (n, t) layout, two 128-partition n-halves
    qrtd = nc.dram_tensor(
        "qrt", [h_per_core, 2, 128, t], FP16, kind="ExternalInput"
    ).ap()
    # qtn: rotated Q, p-major packed (t, n): qtn[h, p, ci*n+m] = QR[h, ci*128+p, m]
    qtnd = nc.dram_tensor(
        "qtn", [h_per_core, 128, nt * n], FP16, kind="ExternalInput"
    ).ap()
    # v: p-major packed like qtn
    vd = nc.dram_tensor(
        "v", [h_per_core, 128, nt * n], FP16, kind="ExternalInput"
    ).ap()
    # o: p-major packed fp16; host unpacks + casts
    od = nc.dram_tensor(
        "o", [h_per_core, 128, nt * n], FP16, kind="ExternalOutput"
    ).ap()

    with tile.TileContext(nc) as tc:
        with (
            tc.tile_pool(name="const", bufs=1) as cpool,
            tc.tile_pool(name="qrt", bufs=4) as qpool,
            tc.tile_pool(name="qtn", bufs=4) as qnpool,
            tc.tile_pool(name="vh", bufs=4) as vpool,
            tc.tile_pool(name="hs", bufs=2) as hspool,
            tc.tile_pool(name="sts", bufs=4) as stspool,
            tc.tile_pool(name="ohs", bufs=2) as ohpool,
            tc.tile_pool(name="hp", bufs=2, space="PSUM") as hpool,
            tc.tile_pool(name="op", bufs=4, space="PSUM") as opool,
            tc.tile_pool(name="sp", bufs=2, space="PSUM") as sppool,
        ):
            # mask for one superchunk's score drain, (s, t') orientation:
            # [0:128]  = strict upper (diag i0), [128:256] = ones (dense),
            # [256:384]= strict upper (diag i1), [384:512] = ones (warmup).
            mask = cpool.tile([128, 512], F32)
            nc.gpsimd.memset(mask, 1.0)
            for c0 in (0, 256):
                nc.gpsimd.affine_select(
                    out=mask[:, c0:c0 + 128],
                    in_=mask[:, c0:c0 + 128],
                    compare_op=mybir.AluOpType.is_ge,
                    fill=0.0,
                    base=-1,
                    pattern=[[1, 128]],
                    channel_multiplier=-1,
                )

            # HAM warmup: dummy fp32 PE activity while head 0's input DMAs
            # are in flight starts the un-throttle clock early.
            for _ in range(3):
                warm = opool.tile([128, 512], F32, tag="op", name="warm")
                nc.tensor.matmul(
                    warm, lhsT=mask[:, 0:128], rhs=mask,
                    start=True, stop=True,
                )

            qrt = {}
            qtn = {}
            vh = {}
            oh = {}
            hp = {}
            hs = {}
            dr = [0]

            def emit_loads(heads):
                """Input DMAs for one head pair.  dma_start occupies the
                ISSUING engine ~0.6us each and blocks in-order on ring-space,
                so inputs are issued only from sync (qrt) and gpsimd (v+qtn)
                — never from vector/scalar, which drain PSUM.  A small first
                descriptor per tensor gets superchunk 0's data in fast; the
                rest ships as one big descriptor each."""
                for h in heads:
                    qrt[h] = [
                        qpool.tile(
                            [128, t], FP16, tag=f"qrt{c}", name=f"qrt{c}_{h}"
                        )
                        for c in range(2)
                    ]
                    qtn[h] = qnpool.tile(
                        [128, nt * n], FP16, tag="qtn", name=f"qtn{h}"
                    )
                    vh[h] = vpool.tile([128, nt * n], FP16, tag="vh", name=f"vh{h}")
                    oh[h] = ohpool.tile(
                        [128, nt * n], FP16, tag=f"oh{h % 2}", name=f"oh{h}"
                    )
                s0q, s0v = slice(0, 512), slice(0, 1024)
                for h in heads:  # first-needed slices, ~128-256KB each
                    for c in range(2):
                        nc.sync.dma_start(out=qrt[h][c][:, s0q], in_=qrtd[h, c][:, s0q])
                    nc.gpsimd.dma_start(out=vh[h][:, s0v], in_=vd[h][:, s0v])
                    nc.gpsimd.dma_start(out=qtn[h][:, s0v], in_=qtnd[h][:, s0v])
                sq, sv = slice(512, t), slice(1024, nt * n)
                for h in heads:  # the bulk
                    for c in range(2):
                        nc.sync.dma_start(out=qrt[h][c][:, sq], in_=qrtd[h, c][:, sq])
                    nc.gpsimd.dma_start(out=vh[h][:, sv], in_=vd[h][:, sv])
                    nc.gpsimd.dma_start(out=qtn[h][:, sv], in_=qtnd[h][:, sv])

            def emit_scores(h, sc):
                """diag(i0) | dense(i1<-i0) | diag(i1) into one PSUM bank,
                single fused mask-mult-cast drain."""
                i0s = slice(sc * 256, sc * 256 + 128)
                i1s = slice(sc * 256 + 128, sc * 256 + 256)
                scs = slice(sc * 256, sc * 256 + 256)
                sp = sppool.tile([128, 384], F32, name="sp")
                first = True
                # diag0|dense share lhsT=QR_i0^T: one 256-wide rhs covers both
                for (osl, ls, rs) in (
                    (slice(0, 256), i0s, scs),
                    (slice(256, 384), i1s, i1s),
                ):
                    for c in range(2):
                        nc.tensor.matmul(
                            sp[:, osl],
                            lhsT=qrt[h][c][:, ls],
                            rhs=qrt[h][c][:, rs],
                            start=first, stop=(c == 1),
                            skip_group_check=True,
                        )
                        first = False
                sts = stspool.tile([128, 384], FP16, name="sts")
                nc.vector.tensor_tensor(
                    out=sts, in0=sp, in1=mask[:, 0:384], op=MULT
                )
                return sts

            def emit_out(h, sc, sts, last_head=False):
                """inter + intra2 for both chunks into one O bank; state for
                both chunks; single O cast + DMA; single H copy."""
                i0c, i1c = 2 * sc, 2 * sc + 1
                m0 = slice(i0c * n, (i0c + 1) * n)
                m1 = slice(i1c * n, (i1c + 1) * n)
                i0s = slice(sc * 256, sc * 256 + 128)
                i1s = slice(sc * 256 + 128, sc * 256 + 256)
                op = opool.tile([128, 512], F32, name="op", tag="op")
                first = True
                if sc > 0:  # inter: O_i += QR_i @ H_{<superchunk}
                    for (osl, csl) in ((slice(0, 256), i0s), (slice(256, 512), i1s)):
                        for c in range(2):
                            nc.tensor.matmul(
                                op[:, osl],
                                lhsT=qrt[h][c][:, csl],
                                rhs=hs[h][:, c * 256:(c + 1) * 256],
                                start=first, stop=False,
                                skip_group_check=True,
                            )
                            first = False
                # intra2: diagonal score blocks @ V
                nc.tensor.matmul(
                    op[:, 0:256], lhsT=sts[:, 0:128], rhs=vh[h][:, m0],
                    start=first, stop=(sc > 0), skip_group_check=True,
                )
                nc.tensor.matmul(
                    op[:, 256:512], lhsT=sts[:, 128:256], rhs=vh[h][:, m0],
                    start=False, stop=False, skip_group_check=True,
                )
                nc.tensor.matmul(
                    op[:, 256:512], lhsT=sts[:, 256:384], rhs=vh[h][:, m1],
                    start=False, stop=True, skip_group_check=True,
                )
                osl = slice(i0c * n, (i1c + 1) * n)
                nc.scalar.copy(out=oh[h][:, osl], in_=op)
                if sc % 2 == 1:  # one output DMA per two superchunks
                    dsl = slice((i0c - 2) * n, (i1c + 1) * n)
                    nc.sync.dma_start(out=od[h][:, dsl], in_=oh[h][:, dsl])
                if sc < ns - 1:
                    # state: H += QR_i0^T V_i0 + QR_i1^T V_i1 (open fp32
                    # accumulation across the head; only the head's first
                    # matmul may use start=True — bank-wide bit clear)
                    for ci, msl in ((i0c, m0), (i1c, m1)):
                        for c in range(2):
                            nc.tensor.matmul(
                                hp[h][:, c * 256:(c + 1) * 256],
                                lhsT=qtn[h][:, ci * n + c * 128: ci * n + (c + 1) * 128],
                                rhs=vh[h][:, msl],
                                start=(ci == 0 and c == 0),
                                stop=(sc == ns - 2 and ci == i1c),
                                skip_group_check=True,
                            )
                    # H-copy on the engine not doing this head's O cast
                    if dr[0] % 2 == 0:
                        nc.vector.tensor_copy(out=hs[h], in_=hp[h])
                    else:
                        nc.scalar.copy(out=hs[h], in_=hp[h])
                dr[0] += 1

            for pair in range(h_per_core // 2):
                heads = (2 * pair, 2 * pair + 1)
                emit_loads(heads)
                for h in heads:
                    hp[h] = hpool.tile([128, 512], F32, tag="hp", name=f"hp{h}")
                    hs[h] = hspool.tile([128, 512], FP16, tag="hs", name=f"hs{h}")
                # scores run one superchunk ahead of the out/state stage so
                # every score-drain has a full stage of PE cover
                cur = {h: emit_scores(h, 0) for h in heads}
                for sc in range(ns):
                    nxt = {}
                    for h in heads:
                        if sc + 1 < ns:
                            nxt[h] = emit_scores(h, sc + 1)
                        emit_out(h, sc, cur[h])
                    cur = nxt

    if waitsplit:
        _split_overloaded_waits(nc)
    return nc


_NC_CACHE = {}


def get_nc(h_per_core=H_PER_CORE, t=T, n=N):
    key = (h_per_core, t, n)
    if key not in _NC_CACHE:
        _NC_CACHE[key] = build_nc(h_per_core, t, n)
    return _NC_CACHE[key]


def make_in_maps(Q, V, n_cores=N_CORES):
    b, nh, t, n = Q.shape
    h_per_core = (b * nh) // n_cores
    nt = t // 128
    qf = np.asarray(Q, dtype=np.float32).reshape(b * nh, t, n)
    vf = np.asarray(V, dtype=np.float32).reshape(b * nh, t, n)
    # RoPE on host in fp32 (input prep, like the layout transposes):
    # qr = q * cos + pairswap(q) * sign-folded-sin
    qsw = qf.reshape(b * nh, t, n // 2, 2)[..., ::-1].reshape(b * nh, t, n)
    cos, sin_a = rope_tables(t, n)
    qr = (qf * cos + qsw * sin_a).astype(HF)
    # (n, t) layout, n-halves split for direct 128-partition DMAs
    qrtb = np.ascontiguousarray(
        qr.transpose(0, 2, 1).reshape(b * nh, 2, 128, t)
    )

    def pmajor(x):  # [h, t, n] -> [h, 128, nt*n] with x[h, ci*128+p, m]
        return np.ascontiguousarray(
            x.reshape(b * nh, nt, 128, n).transpose(0, 2, 1, 3)
        ).reshape(b * nh, 128, nt * n)

    qtnb = pmajor(qr)
    vb = pmajor(vf.astype(HF))
    in_maps = []
    for c in range(n_cores):
        sl = slice(c * h_per_core, (c + 1) * h_per_core)
        in_maps.append(
            {
                "qrt": np.ascontiguousarray(qrtb[sl]),
                "qtn": np.ascontiguousarray(qtnb[sl]),
                "v": np.ascontiguousarray(vb[sl]),
            }
        )
    return in_maps


def unpack_out(outs, b, nh, t, n):
    """[cores][h, 128, nt*n] p-major fp16 -> (b, nh, t, n) fp32."""
    nt = t // 128
    full = np.concatenate(outs, axis=0)  # (b*nh, 128, nt*n)
    full = full.reshape(b * nh, 128, nt, n).transpose(0, 2, 1, 3)
    return np.ascontiguousarray(full).reshape(b, nh, t, n).astype(np.float32)


def kernel(Q, K, V):
    """Full-input entry point: Q, K, V are (B, NH, T, N) float32 numpy arrays.
    K is unused (the module self-keys attention on rotated Q)."""
    Q = np.asarray(Q)
    V = np.asarray(V)
    b, nh, t, n = Q.shape
    nc = get_nc((b * nh) // N_CORES, t, n)
    in_maps = make_in_maps(Q, V, N_CORES)
    res = None
    last_err = None
    for attempt in range(3):  # retry transient device/runtime failures
        try:
            res = run_bass_kernel_spmd(
                nc, in_maps, core_ids=list(range(N_CORES)), trace=False
            )
            break
        except Exception as e:  # e.g. NRT_EXEC_UNIT_UNRECOVERABLE after a
            last_err = e  # wedged prior run; a clean retry usually recovers
            import time as _time

            _time.sleep(2.0 * (attempt + 1))
    if res is None:
        raise last_err
    outs = [res.results[c]["o"] for c in range(N_CORES)]
    return unpack_out(outs, b, nh, t, n)


# revision 35
# speedup vs baseline: 1.5496x; 1.1214x over previous
"""Trainium2 Bass kernel for ContinuousAttention (self-keyed RoPE attention,
strictly-causal masked scores, no softmax).

Reference computation (B=2, NH=16, T=2048, N=256, fp32):
    QR = rope(Q)                      # interleaved-pair RoPE, freqs quantized in pairs
    S  = QR @ QR^T                    # per (b, h); K input is unused by the module
    O  = (S * strict_causal_mask) @ V

Sharding: 32 (b*nh) heads over 8 NeuronCores, 4 heads per core; no
communication.  Each core runs an identical program on its head slice.

v6 design — chunked linear attention (no softmax => scores are linear):
    O_i = QR_i @ H_{<i} + (causal diagonal blocks) @ V,   H += QR_i^T V_i
with a running state H (256x256) accumulated in fp32 PSUM across each head.
PE work is ~2*T*N^2 + ~2.5*T*C*N per head, ~2.7x less than dense-causal.

Superchunks of 256 rows (2 chunks i0, i1) keep the PSUM-drain op count low
(vector/scalar are the only engines that may read PSUM and each drain op has
a few-hundred-ns fixed cost):
  - one [128, 384] score PSUM bank holds diag(i0) | dense(i1,i0) | diag(i1),
    drained by a single mask-multiply-cast (mask = strict|ones|strict),
  - one [128, 512] O PSUM bank holds O_i0 | O_i1, drained by a single cast,
  - one H copy per superchunk; O_i1's missing chunk-i0 term comes from the
    dense block instead of H.
PSUM has_written semantics: start=True clears the accumulate bits of the
WHOLE bank, so only the first matmul targeting a bank uses it; later groups
in the same bank open with start=False (overwrite-where-unset).

Host ships QR pre-rotated in both (n, t) and p-major (t, n) fp16 layouts and
V p-major fp16; all device DMAs are contiguous 2D copies.  Output is fp16
p-major, unpacked on host.  Two heads are interleaved superchunk-by-
superchunk so every drain has a full other-head superchunk of latency cover.
"""

import math
import sys

import numpy as np

if "/opt/trn_rl_repo" not in sys.path:
    sys.path.insert(0, "/opt/trn_rl_repo")

import concourse.bass as bass
import concourse.mybir as mybir
import concourse.tile as tile
from concourse.bass_utils import run_bass_kernel_spmd

B, NH, T, N = 2, 16, 2048, 256
THETA = 2 ** 16
N_CORES = 8
H_PER_CORE = (B * NH) // N_CORES

F32 = mybir.dt.float32
FP16 = mybir.dt.float16
MULT = mybir.AluOpType.mult
HF = np.float16


def _split_overloaded_waits(nc, max_waits=1):
    """walrus in this container rejects >1 sync-wait per instruction; move
    extra waits onto preceding same-engine NoOps (semantically identical)."""
    n_split = 0
    for f in nc.m.functions:
        for bb in f.blocks:
            new_list = []
            changed = False
            for ins in bb.instructions:
                si = getattr(ins, "sync_info", None)
                if si is not None and len(si.on_wait) > max_waits:
                    waits = list(si.on_wait)
                    extra, keep = waits[:-max_waits], waits[-max_waits:]
                    k = 0
                    while extra:
                        chunk, extra = extra[:max_waits], extra[max_waits:]
                        nop = mybir.InstNoOp(
                            name=f"{ins.name}_wsplit{k}", ins=[], outs=[]
                        )
                        nop.engine = ins.engine
                        nop.sync_info = mybir.SyncInfo(on_wait=chunk, on_update=[])
                        new_list.append(nop)
                        k += 1
                    ins.sync_info = mybir.SyncInfo(
                        on_wait=keep, on_update=list(si.on_update)
                    )
                    changed = True
                    n_split += 1
                new_list.append(ins)
            if changed:
                bb.instructions = new_list
    return n_split


def rope_tables(t=T, n=N, dtype=np.float32):
    """cos table and sign-folded sin table, natural (t, n) layout."""
    idx = np.floor(np.arange(n, dtype=dtype) / dtype(2.0)) * dtype(2.0)
    freqs = (
        dtype(1.0) / (dtype(THETA) ** (idx / dtype(n))) / dtype(2.0 * math.pi)
    ).astype(dtype)
    phases = np.arange(t, dtype=dtype)[:, None] * freqs[None, :]
    ph = (phases % dtype(1.0)) * dtype(2.0 * math.pi)
    cos = np.cos(ph).astype(dtype)
    sin = np.sin(ph).astype(dtype)
    sin_a = sin.copy()
    sin_a[:, 0::2] *= dtype(-1.0)  # fold the rotate-pair sign into sin
    return cos, sin_a


def build_nc(h_per_core=H_PER_CORE, t=T, n=N, waitsplit=True):
    assert n == 256 and t % 256 == 0
    nt = t // 128   # 128-row chunks per head (16)
    ns = t // 256   # superchunks per head (8)
    nc = bass.Bass("TRN2", target_bir_lowering=False, debug=False)

    # qrt: rotated Q, (n, t) layout, two 128-partition n-halves
    qrtd = nc.dram_tensor(
        "qrt", [h_per_core, 2, 128, t], FP16, kind="ExternalInput"
    ).ap()
    # qtn: rotated Q, p-major packed (t, n): qtn[h, p, ci*n+m] = QR[h, ci*128+p, m]
    qtnd = nc.dram_tensor(
        "qtn", [h_per_core, 128, nt * n], FP16, kind="ExternalInput"
    ).ap()
    # v: p-major packed like qtn
    vd = nc.dram_tensor(
        "v", [h_per_core, 128, nt * n], FP16, kind="ExternalInput"
    ).ap()
    # o: p-major packed fp16; host unpacks + casts
    od = nc.dram_tensor(
        "o", [h_per_core, 128, nt * n], FP16, kind="ExternalOutput"
    ).ap()

    with tile.TileContext(nc) as tc:
        with (
            tc.tile_pool(name="const", bufs=1) as cpool,
            tc.tile_pool(name="qrt", bufs=4) as qpool,
            tc.tile_pool(name="qtn", bufs=4) as qnpool,
            tc.tile_pool(name="vh", bufs=4) as vpool,
            tc.tile_pool(name="hs", bufs=2) as hspool,
            tc.tile_pool(name="sts", bufs=4) as stspool,
            tc.tile_pool(name="ohs", bufs=2) as ohpool,
            tc.tile_pool(name="hp", bufs=2, space="PSUM") as hpool,
            tc.tile_pool(name="op", bufs=4, space="PSUM") as opool,
            tc.tile_pool(name="sp", bufs=2, space="PSUM") as sppool,
        ):
            # mask for one superchunk's score drain, (s, t') orientation:
            # [0:128]  = strict upper (diag i0), [128:256] = ones (dense),
            # [256:384]= strict upper (diag i1), [384:512] = ones (warmup).
            mask = cpool.tile([128, 512], F32)
            nc.gpsimd.memset(mask, 1.0)
            for c0 in (0, 256):
                nc.gpsimd.affine_select(
                    out=mask[:, c0:c0 + 128],
                    in_=mask[:, c0:c0 + 128],
                    compare_op=mybir.AluOpType.is_ge,
                    fill=0.0,
                    base=-1,
                    pattern=[[1, 128]],
                    channel_multiplier=-1,
                )

            # HAM warmup: dummy fp32 PE activity while head 0's input DMAs
            # are in flight starts the un-throttle clock early.
            for _ in range(3):
                warm = opool.tile([128, 512], F32, tag="op", name="warm")
                nc.tensor.matmul(
                    warm, lhsT=mask[:, 0:128], rhs=mask,
                    start=True, stop=True,
                )

            qrt = {}
            qtn = {}
            vh = {}
            oh = {}
            hp = {}
            hs = {}
            dr = [0]

            def emit_loads(heads):
                """Input DMAs for one head pair.  dma_start occupies the
                ISSUING engine ~0.6us each and blocks in-order on ring-space,
                so inputs are issued only from sync (qrt) and gpsimd (v+qtn)
                — never from vector/scalar, which drain PSUM.  A small first
                descriptor per tensor gets superchunk 0's data in fast; the
                rest ships as one big descriptor each."""
                for h in heads:
                    qrt[h] = [
                        qpool.tile(
                            [128, t], FP16, tag=f"qrt{c}", name=f"qrt{c}_{h}"
                        )
                        for c in range(2)
                    ]
                    qtn[h] = qnpool.tile(
                        [128, nt * n], FP16, tag="qtn", name=f"qtn{h}"
                    )
                    vh[h] = vpool.tile([128, nt * n], FP16, tag="vh", name=f"vh{h}")
                    oh[h] = ohpool.tile(
                        [128, nt * n], FP16, tag=f"oh{h % 2}", name=f"oh{h}"
                    )
                # 4 per-tensor segments, head-interleaved, consumption order:
                # a slice read waits its WHOLE descriptor, so match descriptor
                # boundaries to the compute sweep
                for s in range(4):
                    sq = slice(s * (t // 4), (s + 1) * (t // 4))
                    sv = slice(s * (nt * n // 4), (s + 1) * (nt * n // 4))
                    for h in heads:
                        for c in range(2):
                            nc.sync.dma_start(
                                out=qrt[h][c][:, sq], in_=qrtd[h, c][:, sq]
                            )
                        nc.gpsimd.dma_start(out=vh[h][:, sv], in_=vd[h][:, sv])
                        nc.gpsimd.dma_start(out=qtn[h][:, sv], in_=qtnd[h][:, sv])

            def emit_scores(h, sc):
                """diag(i0) | dense(i1<-i0) | diag(i1) into one PSUM bank,
                single fused mask-mult-cast drain."""
                i0s = slice(sc * 256, sc * 256 + 128)
                i1s = slice(sc * 256 + 128, sc * 256 + 256)
                scs = slice(sc * 256, sc * 256 + 256)
                sp = sppool.tile([128, 384], F32, name="sp")
                first = True
                # diag0|dense share lhsT=QR_i0^T: one 256-wide rhs covers both
                for (osl, ls, rs) in (
                    (slice(0, 256), i0s, scs),
                    (slice(256, 384), i1s, i1s),
                ):
                    for c in range(2):
                        nc.tensor.matmul(
                            sp[:, osl],
                            lhsT=qrt[h][c][:, ls],
                            rhs=qrt[h][c][:, rs],
                            start=first, stop=(c == 1),
                            skip_group_check=True,
                        )
                        first = False
                sts = stspool.tile([128, 384], FP16, name="sts")
                nc.vector.tensor_tensor(
                    out=sts, in0=sp, in1=mask[:, 0:384], op=MULT
                )
                return sts

            def emit_out(h, sc, sts, last_head=False):
                """inter + intra2 for both chunks into one O bank; state for
                both chunks; single O cast + DMA; single H copy."""
                i0c, i1c = 2 * sc, 2 * sc + 1
                m0 = slice(i0c * n, (i0c + 1) * n)
                m1 = slice(i1c * n, (i1c + 1) * n)
                i0s = slice(sc * 256, sc * 256 + 128)
                i1s = slice(sc * 256 + 128, sc * 256 + 256)
                op = opool.tile([128, 512], F32, name="op", tag="op")
                first = True
                if sc > 0:  # inter: O_i += QR_i @ H_{<superchunk}
                    for (osl, csl) in ((slice(0, 256), i0s), (slice(256, 512), i1s)):
                        for c in range(2):
                            nc.tensor.matmul(
                                op[:, osl],
                                lhsT=qrt[h][c][:, csl],
                                rhs=hs[h][:, c * 256:(c + 1) * 256],
                                start=first, stop=False,
                                skip_group_check=True,
                            )
                            first = False
                # intra2: diagonal score blocks @ V
                nc.tensor.matmul(
                    op[:, 0:256], lhsT=sts[:, 0:128], rhs=vh[h][:, m0],
                    start=first, stop=(sc > 0), skip_group_check=True,
                )
                nc.tensor.matmul(
                    op[:, 256:512], lhsT=sts[:, 128:256], rhs=vh[h][:, m0],
                    start=False, stop=False, skip_group_check=True,
                )
                nc.tensor.matmul(
                    op[:, 256:512], lhsT=sts[:, 256:384], rhs=vh[h][:, m1],
                    start=False, stop=True, skip_group_check=True,
                )
                osl = slice(i0c * n, (i1c + 1) * n)
                nc.scalar.copy(out=oh[h][:, osl], in_=op)
                if sc % 2 == 1:  # one output DMA per two superchunks
                    dsl = slice((i0c - 2) * n, (i1c + 1) * n)
                    nc.sync.dma_start(out=od[h][:, dsl], in_=oh[h][:, dsl])
                if sc < ns - 1:
                    # state: H += QR_i0^T V_i0 + QR_i1^T V_i1 (open fp32
                    # accumulation across the head; only the head's first
                    # matmul may use start=True — bank-wide bit clear)
                    for ci, msl in ((i0c, m0), (i1c, m1)):
                        for c in range(2):
                            nc.tensor.matmul(
                                hp[h][:, c * 256:(c + 1) * 256],
                                lhsT=qtn[h][:, ci * n + c * 128: ci * n + (c + 1) * 128],
                                rhs=vh[h][:, msl],
                                start=(ci == 0 and c == 0),
                                stop=(sc == ns - 2 and ci == i1c),
                                skip_group_check=True,
                            )
                    # H-copy on the engine not doing this head's O cast
                    if dr[0] % 2 == 0:
                        nc.vector.tensor_copy(out=hs[h], in_=hp[h])
                    else:
                        nc.scalar.copy(out=hs[h], in_=hp[h])
                dr[0] += 1

            for pair in range(h_per_core // 2):
                heads = (2 * pair, 2 * pair + 1)
                emit_loads(heads)
                for h in heads:
                    hp[h] = hpool.tile([128, 512], F32, tag="hp", name=f"hp{h}")
                    hs[h] = hspool.tile([128, 512], FP16, tag="hs", name=f"hs{h}")
                # scores run one superchunk ahead of the out/state stage so
                # every score-drain has a full stage of PE cover
                cur = {h: emit_scores(h, 0) for h in heads}
                for sc in range(ns):
                    nxt = {}
                    for h in heads:
                        if sc + 1 < ns:
                            nxt[h] = emit_scores(h, sc + 1)
                        emit_out(h, sc, cur[h])
                    cur = nxt

    if waitsplit:
        _split_overloaded_waits(nc)
    return nc


_NC_CACHE = {}


def get_nc(h_per_core=H_PER_CORE, t=T, n=N):
    key = (h_per_core, t, n)
    if key not in _NC_CACHE:
        _NC_CACHE[key] = build_nc(h_per_core, t, n)
    return _NC_CACHE[key]


def make_in_maps(Q, V, n_cores=N_CORES):
    b, nh, t, n = Q.shape
    h_per_core = (b * nh) // n_cores
    nt = t // 128
    qf = np.asarray(Q, dtype=np.float32).reshape(b * nh, t, n)
    vf = np.asarray(V, dtype=np.float32).reshape(b * nh, t, n)
    # RoPE on host in fp32 (input prep, like the layout transposes):
    # qr = q * cos + pairswap(q) * sign-folded-sin
    qsw = qf.reshape(b * nh, t, n // 2, 2)[..., ::-1].reshape(b * nh, t, n)
    cos, sin_a = rope_tables(t, n)
    qr = (qf * cos + qsw * sin_a).astype(HF)
    # (n, t) layout, n-halves split for direct 128-partition DMAs
    qrtb = np.ascontiguousarray(
        qr.transpose(0, 2, 1).reshape(b * nh, 2, 128, t)
    )

    def pmajor(x):  # [h, t, n] -> [h, 128, nt*n] with x[h, ci*128+p, m]
        return np.ascontiguousarray(
            x.reshape(b * nh, nt, 128, n).transpose(0, 2, 1, 3)
        ).reshape(b * nh, 128, nt * n)

    qtnb = pmajor(qr)
    vb = pmajor(vf.astype(HF))
    in_maps = []
    for c in range(n_cores):
        sl = slice(c * h_per_core, (c + 1) * h_per_core)
        in_maps.append(
            {
                "qrt": np.ascontiguousarray(qrtb[sl]),
                "qtn": np.ascontiguousarray(qtnb[sl]),
                "v": np.ascontiguousarray(vb[sl]),
            }
        )
    return in_maps


def unpack_out(outs, b, nh, t, n):
    """[cores][h, 128, nt*n] p-major fp16 -> (b, nh, t, n) fp32."""
    nt = t // 128
    full = np.concatenate(outs, axis=0)  # (b*nh, 128, nt*n)
    full = full.reshape(b * nh, 128, nt, n).transpose(0, 2, 1, 3)
    return np.ascontiguousarray(full).reshape(b, nh, t, n).astype(np.float32)


def kernel(Q, K, V):
    """Full-input entry point: Q, K, V are (B, NH, T, N) float32 numpy arrays.
    K is unused (the module self-keys attention on rotated Q)."""
    Q = np.asarray(Q)
    V = np.asarray(V)
    b, nh, t, n = Q.shape
    nc = get_nc((b * nh) // N_CORES, t, n)
    in_maps = make_in_maps(Q, V, N_CORES)
    res = None
    last_err = None
    for attempt in range(3):  # retry transient device/runtime failures
        try:
            res = run_bass_kernel_spmd(
                nc, in_maps, core_ids=list(range(N_CORES)), trace=False
            )
            break
        except Exception as e:  # e.g. NRT_EXEC_UNIT_UNRECOVERABLE after a
            last_err = e  # wedged prior run; a clean retry usually recovers
            import time as _time

            _time.sleep(2.0 * (attempt + 1))
    if res is None:
        raise last_err
    outs = [res.results[c]["o"] for c in range(N_CORES)]
    return unpack_out(outs, b, nh, t, n)
